# revision 1
# baseline (speedup 1.0000x reference)
"""AutoCov1D Trainium2 kernel (8 NeuronCores, data-parallel over batch).

Math: for window n (stride 8, width 64), with X1 = X[:, :-64], X2 = X[:, 64:]:
  p1 = einsum('bnw,wdc', X1win, Wgt); p2 likewise with X2win
  out = mean_d(p1c * p2c) + bias   (p*c centered over d)

Exact simplifications used here:
  1. Centering over d is linear in the weight, so pre-center the weight:
     Wtil = (W - mean_d W) / sqrt(D); then no mean terms remain.
  2. X2 windows are X1 windows shifted by 8 window indices (64 = 8*stride),
     so ONE projection P[b,m,:] = sum_w X[b, 8m+w] * Wtil[w,:] over m=0..504
     serves both operands:  out[b,n,c] = sum_d P[b,n,d,c]*P[b,n+8,d,c] + bias.

Per-core pipeline (B_shard=4), bf16 compute / fp32 accumulate:
  - weight-stationary bf16 matmuls, PSUM tiles P[(d4,c32)=128 partitions, m]
    (4 latent dims x 32 channels per tile, 8 accumulation quads dq)
  - ACT (+some DVE) evacuates PSUM -> SBUF bf16
  - DVE forms shifted products P[n]*P[n+8] (bf16 2x mode)
  - TensorE selector matmuls (K=128 -> M=32) reduce the 4 in-tile latent
    dims and accumulate the 8 quads in PSUM fp32 (exact reduction)
  - DVE adds bias and writes the fp32 output tile
"""

import sys

import numpy as np

if "/opt/trn_rl_repo" not in sys.path:
    sys.path.insert(0, "/opt/trn_rl_repo")

_B, _T, _W, _D, _C = 32, 4096, 64, 32, 128
_NCORES = 8
_BSH = _B // _NCORES  # 4
_M = 505  # projection windows per batch row
_N = 497  # output windows per batch row
_S = 8  # stride
_MM = 506  # matmul free dim (last col unused; kept even)
_XLEN = _MM * _S  # 4048, padded per-partition X span

_NC_CACHE = None


def _build_nc():
    import concourse.bass as bass
    import concourse.tile as tile
    from concourse import bacc, mybir
    from contextlib import ExitStack

    f32 = mybir.dt.float32
    bf16 = mybir.dt.bfloat16

    nc = bacc.Bacc(None, target_bir_lowering=False)
    x = nc.declare_dram_parameter("xsh", [_BSH, _W, _XLEN], bf16, isOutput=False)
    # wt[w, dq, cb, dd*32+cc] = Wtil[w, 4*dq+dd, 32*cb+cc]
    wt = nc.declare_dram_parameter("wt", [_W, 8, 4, _C], bf16, isOutput=False)
    sel = nc.declare_dram_parameter("sel", [_C, 32], bf16, isOutput=False)
    bias = nc.declare_dram_parameter("bias", [_C, 1], f32, isOutput=False)
    out = nc.declare_dram_parameter("out", [_BSH, _C, _N], f32, isOutput=True)

    with ExitStack() as ctx:
        tc = ctx.enter_context(tile.TileContext(nc))
        singles = ctx.enter_context(tc.tile_pool(name="singles", bufs=1))
        xpool = ctx.enter_context(tc.tile_pool(name="xpool", bufs=2))
        psp = ctx.enter_context(tc.tile_pool(name="psp", bufs=3, space="PSUM"))
        covp = ctx.enter_context(tc.tile_pool(name="covp", bufs=2, space="PSUM"))
        evacp = ctx.enter_context(tc.tile_pool(name="evacp", bufs=6))
        prodp = ctx.enter_context(tc.tile_pool(name="prodp", bufs=6))
        outp = ctx.enter_context(tc.tile_pool(name="outp", bufs=2))

        wt_tiles = []
        for g in range(2):
            wtile = singles.tile([_W, 4, 4, _C], bf16, tag=f"wtg{g}")
            nc.sync.dma_start(out=wtile, in_=wt[:, 4 * g : 4 * g + 4, :, :])
            wt_tiles.append(wtile)
        sel_sb = singles.tile([_C, 32], bf16)
        nc.sync.dma_start(out=sel_sb, in_=sel[:, :])
        bias_sb = singles.tile([_C, 1], f32)
        nc.sync.dma_start(out=bias_sb, in_=bias[:, :])

        for b in range(_BSH):
            xsh = xpool.tile([_W, _XLEN], bf16)
            for pc in range(4):
                nc.sync.dma_start(
                    out=xsh[16 * pc : 16 * pc + 16, :],
                    in_=x[b, 16 * pc : 16 * pc + 16, :],
                )
            # strided view: rhs[w, m] = X[b, 8m + w]
            xview = xsh.rearrange("p (m s) -> p m s", s=_S)
            cov = covp.tile([_C, _N], f32)
            ev_i = 0
            for dq in range(8):
                for cp in range(2):
                    ps = psp.tile([_C, 2, 512], f32)
                    for j in range(2):
                        cb = 2 * cp + j
                        nc.tensor.matmul(
                            ps[:, j, 0:_MM],
                            lhsT=wt_tiles[dq // 4][:, dq % 4, cb, :],
                            rhs=xview[:, 0:_MM, 0],
                            start=True,
                            stop=True,
                        )
                    ev = evacp.tile([_C, 2, _MM], bf16)
                    if ev_i % 8 < 6:
                        nc.scalar.copy(out=ev[:, :, :], in_=ps[:, :, 0:_MM])
                    else:
                        nc.vector.tensor_copy(ev[:, :, :], ps[:, :, 0:_MM])
                    ev_i += 1
                    for j in range(2):
                        cb = 2 * cp + j
                        pr = prodp.tile([_C, _N], bf16)
                        nc.vector.tensor_mul(
                            pr[:, :], ev[:, j, 0:_N], ev[:, j, _S:_M]
                        )
                        nc.tensor.matmul(
                            cov[32 * cb : 32 * cb + 32, :],
                            lhsT=sel_sb[:, :],
                            rhs=pr[:, :],
                            start=(dq == 0),
                            stop=(dq == 7),
                            tile_position=(0, 32 * cb),
                        )
            ot = outp.tile([_C, _N], f32)
            nc.vector.tensor_scalar_add(ot[:, :], cov[:, :], bias_sb[:, 0:1])
            nc.sync.dma_start(out=out[b], in_=ot[:, :])
    nc.finalize()
    return nc


def _prep_inputs(X, weight, bias):
    import ml_dtypes

    X = np.asarray(X, dtype=np.float32)
    weight = np.asarray(weight, dtype=np.float32)
    bias = np.asarray(bias, dtype=np.float32)

    wtil = (weight - weight.mean(axis=1, keepdims=True)) / np.sqrt(np.float32(_D))
    # regroup to [w, dq, cb, dd*32+cc]
    wsel = (
        wtil.reshape(_W, 8, 4, 4, 32)  # w, dq, dd, cb, cc
        .transpose(0, 1, 3, 2, 4)  # w, dq, cb, dd, cc
        .reshape(_W, 8, 4, _C)
    )
    wsel = np.ascontiguousarray(wsel).astype(ml_dtypes.bfloat16)

    # xsh[b, w, i] = X[b, w + i]  (zero-padded past T)
    xsh = np.zeros((_B, _W, _XLEN), dtype=np.float32)
    for w in range(_W):
        n = min(_XLEN, _T - w)
        xsh[:, w, :n] = X[:, w : w + n]
    xsh = xsh.astype(ml_dtypes.bfloat16)

    selm = np.zeros((_C, 32), dtype=np.float32)
    for p in range(_C):
        selm[p, p % 32] = 1.0
    selm = selm.astype(ml_dtypes.bfloat16)

    bias2 = np.ascontiguousarray(bias.reshape(_C, 1))

    in_maps = []
    for k in range(_NCORES):
        in_maps.append(
            {
                "xsh": np.ascontiguousarray(xsh[k * _BSH : (k + 1) * _BSH]),
                "wt": wsel,
                "sel": selm,
                "bias": bias2,
            }
        )
    return in_maps


def get_nc():
    global _NC_CACHE
    if _NC_CACHE is None:
        _NC_CACHE = _build_nc()
    return _NC_CACHE


def run(X, weight, bias, trace=False, tmpdir=None):
    """Returns (full_output, BassKernelResults)."""
    from concourse.bass_utils import run_bass_kernel_spmd

    nc = get_nc()
    in_maps = _prep_inputs(X, weight, bias)
    res = run_bass_kernel_spmd(
        nc, in_maps, core_ids=list(range(_NCORES)), trace=trace, tmpdir=tmpdir
    )
    parts = [res.results[i]["out"].transpose(0, 2, 1) for i in range(_NCORES)]
    full = np.ascontiguousarray(np.concatenate(parts, axis=0), dtype=np.float32)
    return full, res


def kernel(X, weight, bias):
    full, _ = run(X, weight, bias)
    return full



# revision 4
# speedup vs baseline: 2.2852x; 2.2852x over previous
"""AutoCov1D Trainium2 kernel (8 NeuronCores, data-parallel over batch).

Math: for window n (stride 8, width 64), with X1 = X[:, :-64], X2 = X[:, 64:]:
  p1 = einsum('bnw,wdc', X1win, Wgt); p2 likewise with X2win
  out = mean_d(p1c * p2c) + bias   (p*c centered over d)

Exact simplifications:
  1. Centering over d is linear in the weight, so pre-center the weight:
     Wtil = (W - mean_d W) / sqrt(D); then no mean terms remain.
  2. X2 windows are X1 windows shifted by 8 window indices (64 = 8*stride),
     so ONE projection P[b,m,:] = sum_w X[b, 8m+w] * Wtil[w,:] over m=0..504
     serves both operands:  out[b,n,c] = sum_d P[b,n,d,c]*P[b,n+8,d,c] + bias.

v2 layout changes vs v1:
  - X is pre-decimated on the host: xdt[w, b, j] = X[b, 8j + w], so the
    projection matmul rhs is CONTIGUOUS in SBUF (v1 read stride-8, ~2x
    slower PE columns) and X DMA volume drops 8x.
  - PE instruction stream is phase-grouped per batch row: 32 projection
    matmuls (b), then 32 selector matmuls (b-1, products all ready), which
    keeps the tensor engine continuously busy (p-state ramps to 2.4 GHz)
    and lets selector matmuls share ldweights-friendly cb-outer order.
  - PSUM evacuation (fp32 -> bf16) splits between ScalarE and VectorE;
    products are one DVE op per dq (4 cb slices in a 3D access pattern).
"""

import sys

import numpy as np

if "/opt/trn_rl_repo" not in sys.path:
    sys.path.insert(0, "/opt/trn_rl_repo")

_B, _T, _W, _D, _C = 32, 4096, 64, 32, 128
_NCORES = 8
_BSH = _B // _NCORES  # 4
_M = 505  # projection windows per batch row
_N = 497  # output windows per batch row
_S = 8  # stride, also window shift in m-space
_MM = 506  # projection matmul free dim (even; last col pad)
_JW = 512  # padded xdt column count

_NC_CACHE = None


def _build_nc():
    import concourse.bass as bass
    import concourse.tile as tile
    from concourse import bacc, mybir
    from contextlib import ExitStack

    f32 = mybir.dt.float32
    bf16 = mybir.dt.bfloat16

    nc = bacc.Bacc(None, target_bir_lowering=False)
    # xdt[w, b, j] = X[b, 8j + w] (zero past the end)
    xdt = nc.declare_dram_parameter("xdt", [_W, _BSH, _JW], bf16, isOutput=False)
    # wt[w, dq, cb, dd*32+cc] = Wtil[w, 4*dq+dd, 32*cb+cc]
    wt = nc.declare_dram_parameter("wt", [_W, 8, 4, _C], bf16, isOutput=False)
    sel = nc.declare_dram_parameter("sel", [_C, 32], bf16, isOutput=False)
    bias = nc.declare_dram_parameter("bias", [_C, 1], f32, isOutput=False)
    out = nc.declare_dram_parameter("out", [_BSH, _C, _N], f32, isOutput=True)

    with ExitStack() as ctx:
        tc = ctx.enter_context(tile.TileContext(nc))
        singles = ctx.enter_context(tc.tile_pool(name="singles", bufs=1))
        psp = ctx.enter_context(tc.tile_pool(name="psp", bufs=3, space="PSUM"))
        covp = ctx.enter_context(tc.tile_pool(name="covp", bufs=2, space="PSUM"))
        evacp = ctx.enter_context(tc.tile_pool(name="evacp", bufs=3))
        prodp = ctx.enter_context(tc.tile_pool(name="prodp", bufs=2))
        outp = ctx.enter_context(tc.tile_pool(name="outp", bufs=2))

        wt_sb = singles.tile([_W, 8, 4, _C], bf16)
        nc.sync.dma_start(out=wt_sb, in_=wt[:, :, :, :])
        xdt_sb = singles.tile([_W, _BSH, _JW], bf16)
        nc.sync.dma_start(out=xdt_sb, in_=xdt[:, :, :])
        sel_sb = singles.tile([_C, 32], bf16)
        nc.sync.dma_start(out=sel_sb, in_=sel[:, :])
        bias_sb = singles.tile([_C, 1], f32)
        nc.sync.dma_start(out=bias_sb, in_=bias[:, :])

        def sel_phase(bb, pr):
            cov = covp.tile([_C, _JW], f32)
            for cb in range(4):
                for dq in range(8):
                    nc.tensor.matmul(
                        cov[32 * cb : 32 * cb + 32, 0:_N],
                        lhsT=sel_sb[:, :],
                        rhs=pr[:, dq, cb, 0:_N],
                        start=(dq == 0),
                        stop=(dq == 7),
                        tile_position=(0, 32 * cb),
                    )
            ot = outp.tile([_C, _N], f32)
            nc.vector.tensor_scalar_add(ot[:, :], cov[:, 0:_N], bias_sb[:, 0:1])
            nc.sync.dma_start(out=out[bb], in_=ot[:, :])

        pr_tiles = [None] * _BSH

        ev_i = 0
        for b in range(_BSH):
            pr = prodp.tile([_C, 8, 4, _JW], bf16)
            pr_tiles[b] = pr
            for dq in range(8):
                ev = evacp.tile([_C, 4, _JW], bf16)
                for cp in range(2):
                    ps = psp.tile([_C, 2, 512], f32)
                    for j in range(2):
                        cb = 2 * cp + j
                        nc.tensor.matmul(
                            ps[:, j, 0:_MM],
                            lhsT=wt_sb[:, dq, cb, :],
                            rhs=xdt_sb[:, b, 0:_MM],
                            start=True,
                            stop=True,
                        )
                    # evacuate fp32 PSUM -> bf16 SBUF (1x rate on either
                    # engine; split ~4:1 ACT:DVE to balance engine load)
                    dst = ev[:, 2 * cp : 2 * cp + 2, 0:_MM]
                    if ev_i % 5 == 4:
                        nc.vector.tensor_copy(dst, ps[:, :, 0:_MM])
                    else:
                        nc.scalar.copy(out=dst, in_=ps[:, :, 0:_MM])
                    ev_i += 1
                # shifted product for all 4 cb slices of this dq in one op
                nc.vector.tensor_mul(
                    pr[:, dq, :, 0:_N], ev[:, :, 0:_N], ev[:, :, _S : _S + _N]
                )
            if b > 0:
                sel_phase(b - 1, pr_tiles[b - 1])
                pr_tiles[b - 1] = None
        sel_phase(_BSH - 1, pr_tiles[_BSH - 1])
    nc.finalize()
    return nc


def _prep_inputs(X, weight, bias):
    import ml_dtypes

    X = np.asarray(X, dtype=np.float32)
    weight = np.asarray(weight, dtype=np.float32)
    bias = np.asarray(bias, dtype=np.float32)

    wtil = (weight - weight.mean(axis=1, keepdims=True)) / np.sqrt(np.float32(_D))
    # regroup to [w, dq, cb, dd*32+cc]
    wsel = (
        wtil.reshape(_W, 8, 4, 4, 32)  # w, dq, dd, cb, cc
        .transpose(0, 1, 3, 2, 4)  # w, dq, cb, dd, cc
        .reshape(_W, 8, 4, _C)
    )
    wsel = np.ascontiguousarray(wsel).astype(ml_dtypes.bfloat16)

    # xdt[b, w, j] = X[b, 8j + w] for j < 506 (zero-padded past T)
    Xp = np.zeros((_B, _T + 64), dtype=np.float32)
    Xp[:, :_T] = X
    xd = np.zeros((_B, _W, _JW), dtype=np.float32)
    for w in range(_W):
        xd[:, w, :_MM] = Xp[:, w : w + 8 * _MM : 8]
    xd = xd.astype(ml_dtypes.bfloat16)

    selm = np.zeros((_C, 32), dtype=np.float32)
    for p in range(_C):
        selm[p, p % 32] = 1.0
    selm = selm.astype(ml_dtypes.bfloat16)

    bias2 = np.ascontiguousarray(bias.reshape(_C, 1))

    in_maps = []
    for k in range(_NCORES):
        xdt_k = np.ascontiguousarray(
            xd[k * _BSH : (k + 1) * _BSH].transpose(1, 0, 2)
        )
        in_maps.append(
            {
                "xdt": xdt_k,
                "wt": wsel,
                "sel": selm,
                "bias": bias2,
            }
        )
    return in_maps


def get_nc():
    global _NC_CACHE
    if _NC_CACHE is None:
        _NC_CACHE = _build_nc()
    return _NC_CACHE


def run(X, weight, bias, trace=False, tmpdir=None):
    """Returns (full_output, BassKernelResults)."""
    from concourse.bass_utils import run_bass_kernel_spmd

    nc = get_nc()
    in_maps = _prep_inputs(X, weight, bias)
    res = run_bass_kernel_spmd(
        nc, in_maps, core_ids=list(range(_NCORES)), trace=trace, tmpdir=tmpdir
    )
    parts = [res.results[i]["out"].transpose(0, 2, 1) for i in range(_NCORES)]
    full = np.ascontiguousarray(np.concatenate(parts, axis=0), dtype=np.float32)
    return full, res


def kernel(X, weight, bias):
    full, _ = run(X, weight, bias)
    return full
